# revision 2
# baseline (speedup 1.0000x reference)
"""Trainium2 Bass kernel for nn_DiffeqSolver: fixed-grid RK4 neural-ODE
integration of f(y) = conv2(tanh(conv1(y))) with 3x3 SAME convs, C=128.

Sharding: data-parallel over batch B=16 across 8 cores (2 images/core).
Each core integrates its own trajectories; weights replicated.

Conv-as-matmul: channels (128) live on the partition axis; a 3x3 SAME conv
is 9 shifted-tap matmuls accumulating in PSUM, reading a zero-padded
[128, img, 34, 34] activation buffer with windowed access patterns.
Matmul dtype is float32r (TF32-like: full-rate on PE, ~1e-4 relative
per-conv error); RK4 state stays fp32 on DVE.
"""
import sys

if '/opt/trn_rl_repo' not in sys.path:
    sys.path.insert(0, '/opt/trn_rl_repo')

import numpy as np

import concourse.bass as bass
import concourse.tile as tile
from concourse import bacc, mybir
from concourse.bass_utils import run_bass_kernel_spmd

F32 = mybir.dt.float32
F32R = mybir.dt.float32r
MULT = mybir.AluOpType.mult
ADD = mybir.AluOpType.add
Tanh = mybir.ActivationFunctionType.Tanh
Identity = mybir.ActivationFunctionType.Identity

B, C, H, W = 16, 128, 32, 32
T = 25
NCORES = 8
IPC = B // NCORES            # images per core
HP, WP = H + 2, W + 2        # padded spatial
NCHUNK = H // 16             # 512-column chunks per image


def _build(dts, b2_nonzero):
    """Build + compile the per-core Bass program for len(dts) RK4 steps."""
    nsteps = len(dts)
    nc = bacc.Bacc("TRN2", target_bir_lowering=False, debug=False,
                   num_devices=NCORES)

    x_d = nc.dram_tensor("x0", [C, IPC, H, W], F32, kind="ExternalInput")
    w1_d = nc.dram_tensor("w1t", [C, 9 * C], F32, kind="ExternalInput")
    w2_d = nc.dram_tensor("w2t", [C, 9 * C], F32, kind="ExternalInput")
    b1_d = nc.dram_tensor("b1c", [C, 1], F32, kind="ExternalInput")
    b2_d = nc.dram_tensor("b2c", [C, 1], F32, kind="ExternalInput")
    out_d = nc.dram_tensor("out", [nsteps, C, IPC, H, W], F32,
                           kind="ExternalOutput")

    with tile.TileContext(nc) as tc:
        with (
            tc.tile_pool(name="persist", bufs=1) as pp,
            tc.tile_pool(name="psum1", bufs=4, space="PSUM") as ps1,
            tc.tile_pool(name="psum2", bufs=4, space="PSUM") as ps2,
            tc.tile_pool(name="bias", bufs=4) as bp,
        ):
            # persistent state
            Y = pp.tile([C, IPC, H, W], F32, tag="Y")
            ACC = pp.tile([C, IPC, H, W], F32, tag="ACC")
            YB = pp.tile([C, IPC, HP, WP], F32R, tag="YB")
            YT0 = pp.tile([C, IPC, HP, WP], F32R, tag="YT0")
            YT1 = pp.tile([C, IPC, HP, WP], F32R, tag="YT1")
            U = pp.tile([C, IPC, HP, WP], F32R, tag="U")
            W1s = pp.tile([C, 9 * C], F32, tag="W1s")
            W2s = pp.tile([C, 9 * C], F32, tag="W2s")
            W1r = pp.tile([C, 9 * C], F32R, tag="W1r")
            W2r = pp.tile([C, 9 * C], F32R, tag="W2r")
            b1t = pp.tile([C, 1], F32, tag="b1t")
            b2t = pp.tile([C, 1], F32, tag="b2t")

            # loads
            nc.sync.dma_start(Y[:], x_d[:])
            nc.sync.dma_start(W1s[:], w1_d[:])
            nc.sync.dma_start(W2s[:], w2_d[:])
            nc.sync.dma_start(b1t[:], b1_d[:])
            nc.sync.dma_start(b2t[:], b2_d[:])
            nc.vector.tensor_copy(W1r[:], W1s[:])
            nc.vector.tensor_copy(W2r[:], W2s[:])

            # zero padded buffers once (borders stay zero forever).
            # memset can't emit f32r, so round zeros through a DVE copy.
            Z = pp.tile([C, IPC, HP, WP], F32, tag="Z")
            nc.vector.memset(Z[:], 0.0)
            nc.vector.tensor_copy(YB[:], Z[:])
            nc.vector.tensor_copy(YT0[:], Z[:])
            nc.vector.tensor_copy(YT1[:], Z[:])
            nc.vector.tensor_copy(U[:], Z[:])

            # YB interior <- Y
            nc.vector.tensor_copy(YB[:, :, 1:H + 1, 1:W + 1], Y[:])

            def conv(src, wr, on_chunk):
                """3x3 SAME conv of padded src via 9-tap matmul accumulation.
                on_chunk(psum_tile, b, h) consumes each [C,16,W] chunk."""
                for b in range(IPC):
                    for h in range(NCHUNK):
                        p = (ps1 if on_chunk.__name__ == "tanh_chunk" else ps2
                             ).tile([C, 16, W], F32,
                                    tag="p1" if on_chunk.__name__ == "tanh_chunk"
                                    else "p2")
                        r0 = 16 * h
                        for ky in range(3):
                            for kx in range(3):
                                tap = ky * 3 + kx
                                rhs = src[:, b, r0 + ky:r0 + ky + 16,
                                          kx:kx + W]
                                nc.tensor.matmul(
                                    p[:], wr[:, tap * C:(tap + 1) * C], rhs,
                                    start=(tap == 0), stop=(tap == 8))
                        on_chunk(p, b, h)

            for step in range(nsteps):
                dt = float(dts[step])
                # scale applied to k_e when forming the next probe state
                probe_scale = [dt / 2.0, dt / 2.0, dt, None]
                # weight of k_e in the final accumulator
                acc_w = [dt / 6.0, dt / 3.0, dt / 3.0, dt / 6.0]

                srcs = [YB, YT0, YT1, YT0]
                for e in range(4):
                    src = srcs[e]
                    dst = srcs[e + 1] if e < 3 else None

                    def tanh_chunk(p, b, h):
                        nc.scalar.activation(
                            U[:, b, 1 + 16 * h:17 + 16 * h, 1:W + 1], p[:],
                            Tanh, bias=b1t[:, 0:1])

                    conv(src, W1r, tanh_chunk)

                    def k_chunk(p, b, h):
                        r0 = 16 * h
                        acc_c = ACC[:, b, r0:r0 + 16, :]
                        y_c = Y[:, b, r0:r0 + 16, :]
                        kin = p[:]
                        if b2_nonzero:
                            pb = bp.tile([C, 16, W], F32, tag="pb")
                            nc.scalar.activation(pb[:], p[:], Identity,
                                                 bias=b2t[:, 0:1])
                            kin = pb[:]
                        if e == 0:
                            nc.vector.tensor_scalar_mul(acc_c, kin, acc_w[0])
                        else:
                            nc.vector.scalar_tensor_tensor(
                                acc_c, kin, acc_w[e], acc_c, op0=MULT, op1=ADD)
                        if e < 3:
                            yt_c = dst[:, b, 1 + r0:17 + r0, 1:W + 1]
                            nc.vector.scalar_tensor_tensor(
                                yt_c, kin, probe_scale[e], y_c,
                                op0=MULT, op1=ADD)

                    conv(U, W2r, k_chunk)

                # y <- y + acc ; refresh f32r copy ; snapshot
                nc.vector.tensor_add(Y[:], Y[:], ACC[:])
                nc.vector.tensor_copy(YB[:, :, 1:H + 1, 1:W + 1], Y[:])
                nc.sync.dma_start(out_d[step], Y[:])

    nc.compile()
    return nc


_CACHE = {}


def _get_program(dts, b2_nonzero):
    key = (tuple(np.asarray(dts, dtype=np.float32).tolist()), b2_nonzero)
    if key not in _CACHE:
        _CACHE[key] = _build(np.asarray(dts, dtype=np.float32), b2_nonzero)
    return _CACHE[key]


def _run(first_point, time_steps_to_predict, W1, b1, W2, b2, trace=False):
    first_point = np.ascontiguousarray(first_point, dtype=np.float32)
    tgrid = np.asarray(time_steps_to_predict, dtype=np.float32)
    dts = np.diff(tgrid)
    nsteps = len(dts)
    b2 = np.asarray(b2, dtype=np.float32)
    b2_nonzero = bool(np.any(b2 != 0))

    nc = _get_program(dts, b2_nonzero)

    w1t = np.ascontiguousarray(
        np.asarray(W1, dtype=np.float32).transpose(1, 2, 3, 0).reshape(C, 9 * C))
    w2t = np.ascontiguousarray(
        np.asarray(W2, dtype=np.float32).transpose(1, 2, 3, 0).reshape(C, 9 * C))
    b1c = np.ascontiguousarray(np.asarray(b1, dtype=np.float32).reshape(C, 1))
    b2c = np.ascontiguousarray(b2.reshape(C, 1))

    in_maps = []
    for i in range(NCORES):
        x0 = np.ascontiguousarray(
            first_point[IPC * i:IPC * (i + 1)].transpose(1, 0, 2, 3))
        in_maps.append({"x0": x0, "w1t": w1t, "w2t": w2t,
                        "b1c": b1c, "b2c": b2c})

    rr = run_bass_kernel_spmd(nc, in_maps, list(range(NCORES)), trace=trace)

    full = np.empty((B, nsteps + 1, C, H, W), dtype=np.float32)
    full[:, 0] = first_point
    for i in range(NCORES):
        o = rr.results[i]["out"]            # [nsteps, C, IPC, H, W]
        full[IPC * i:IPC * (i + 1), 1:] = o.transpose(2, 0, 1, 3, 4)
    return full, rr.exec_time_ns


def kernel(first_point, time_steps_to_predict, W1, b1, W2, b2):
    out, _ = _run(first_point, time_steps_to_predict, W1, b1, W2, b2)
    return out
